# revision 29
# baseline (speedup 1.0000x reference)
"""Trainium2 Bass kernel for nn_Block (ragged transformer block), v3.

B=2, T=2048, D=768, H=12, DH=64, FF=3072.

Sharding: 8 cores = 2 batches x 4 token-blocks of 512. Each core computes
Q/K/V for ITS OWN 512-token block only; K^T and V' are AllGathered (bf16,
HBM-HBM, groups [0-3] / [4-7]) so the 4 cores of a batch share the full
sequence without recomputing it.

The gather is software-pipelined ONE REP AHEAD: during rep n the kernel
computes the next rep's K/V-own block and issues its AllGather right after
the Q projection, so the collective's wall time (~order 100us) overlaps the
rest of rep n (attention, FFN) and rep n+1 only has to land the already-
gathered bytes. gpsimd runs ONLY the collectives (it executes in order, so
any compute placed there would serialize behind a collective); all
element-wise work is on DVE, softmax denominator broadcast is a PE
ones-matmul.

All matmuls run in bf16 (same PE rate as float32r, half the HBM/SBUF
traffic; residuals and LN statistics stay in fp32). Padded key rows are
zeroed via vmask (folded into V'); padded query rows are only masked at the
final LN2 output - identical to the reference (dead-row intermediates never
reach live rows). Act-table loads (Exp/Sqrt/Gelu live in different table
sets) are prefetched with dummy activations placed between real users.
"""
import sys
for _p in ("/opt/trn_rl_repo", "/root/.axon_site/_ro/trn_rl_repo"):
    if _p not in sys.path:
        sys.path.append(_p)

from contextlib import ExitStack
import numpy as np

B, T, D, H, DH, FF = 2, 2048, 768, 12, 64, 3072
M = 512            # tokens per core
DC = 6             # D / 128
FC = 24            # FF / 128
NKC = 16           # T / 128
VW = H * (DH + 1)  # 780: V' width (64 cols + 1 ones-bias col per head)
KW = DC * M        # k columns in the combined gather bounce
KVW = KW + 4 * VW  # + v columns
EPS = 1e-5

_STATE: dict = {}


def _build_program(reps=1):
    import concourse.mybir as mybir
    import concourse.tile as tile
    from concourse import bacc

    F32 = mybir.dt.float32
    F32R = mybir.dt.float32r
    BF16 = mybir.dt.bfloat16
    AF = mybir.ActivationFunctionType
    OP = mybir.AluOpType

    nc = bacc.Bacc("TRN2", target_bir_lowering=False, debug=False, num_devices=8)

    def din(name, shape, dt=BF16):
        return nc.dram_tensor(name, shape, dt, kind="ExternalInput").ap()

    xTq = din("xTq", [D, M])             # own block, bf16, feature-major
    wq = din("wq", [D, D])
    wk = din("wk", [D, D])
    bq_pc = din("bq_pc", [128, DC], F32)
    bk_pc = din("bk_pc", [128, DC], F32)
    wv = din("wv", [D, VW])
    bv = din("bv", [1, VW])
    wproj = din("wproj", [D, D])
    wfc = din("wfc", [D, FF])
    wout = din("wout", [FF, D])
    onesr = din("onesr", [1, M])
    ones128 = din("ones128", [128, 128], F32R)
    ones64 = din("ones64", [1, 64], F32R)
    bprj = din("bprj", [128, DC], F32)
    bfc = din("bfc", [128, FC], F32)
    bout = din("bout", [128, DC], F32)
    l1g = din("l1g", [128, DC], F32)
    l1b = din("l1b", [128, DC], F32)
    l2g = din("l2g", [128, DC], F32)
    l2b = din("l2b", [128, DC], F32)
    vmask = din("vmask", [128, 4], F32)   # own-block key validity per chunk
    epsc = din("epsc", [128, 1], F32)
    rowmask = din("rowmask", [128, M], F32)

    hT = nc.dram_tensor("hT", [D, M], F32, kind="ExternalOutput").ap()

    xTq_r = xTq.rearrange("(c p) n -> p c n", p=128)
    wq_r = wq.rearrange("(c p) n -> p c n", p=128)
    wk_r = wk.rearrange("(c p) n -> p c n", p=128)
    wv_r = wv.rearrange("(c p) n -> p c n", p=128)
    wproj_r = wproj.rearrange("(c p) n -> p c n", p=128)
    wfc_r = wfc.rearrange("(c p) n -> p c n", p=128)
    wout_r = wout.rearrange("(c p) n -> p c n", p=128)
    hT_r = hT.rearrange("(c p) n -> c p n", p=128)

    GROUPS = [[0, 1, 2, 3], [4, 5, 6, 7]]

    with tile.TileContext(nc) as tc, ExitStack() as ctx:
        const = ctx.enter_context(tc.tile_pool(name="const", bufs=1))
        big = ctx.enter_context(tc.tile_pool(name="big", bufs=1))
        own = ctx.enter_context(tc.tile_pool(name="own", bufs=3))
        xq2 = ctx.enter_context(tc.tile_pool(name="xq2", bufs=2))
        dram = ctx.enter_context(tc.tile_pool(name="dram", bufs=3,
                                              space="DRAM"))

        consts = {}

        def cload(name, shape, dt, src):
            t = const.tile(shape, dt, tag=name, name=name + "_t")
            nc.sync.dma_start(out=t, in_=src)
            return t

        def cloads():
            consts["onesr"] = cload("onesr", [1, M], BF16, onesr)
            consts["ones128"] = cload("ones128", [128, 128], F32R, ones128)
            consts["ones64"] = cload("ones64", [1, 64], F32R, ones64)
            consts["vmask"] = cload("vmask", [128, 4], F32, vmask)
            consts["epsc"] = cload("epsc", [128, 1], F32, epsc)
            consts["rowmask"] = cload("rowmask", [128, M], F32, rowmask)
            consts["bprj"] = cload("bprj", [128, DC], F32, bprj)
            consts["bfc"] = cload("bfc", [128, FC], F32, bfc)
            consts["bout"] = cload("bout", [128, DC], F32, bout)
            consts["l1g"] = cload("l1g", [128, DC], F32, l1g)
            consts["l1b"] = cload("l1b", [128, DC], F32, l1b)
            consts["l2g"] = cload("l2g", [128, DC], F32, l2g)
            consts["l2b"] = cload("l2b", [128, DC], F32, l2b)
            consts["bq"] = cload("bq_pc", [128, DC], F32, bq_pc)
            consts["bk"] = cload("bk_pc", [128, DC], F32, bk_pc)
            consts["bv"] = cload("bv", [1, VW], BF16, bv)

        def prefetch_act(pool, tag, func):
            d = pool.tile([1, 1], F32, tag=tag, name="dum_" + tag)
            nc.vector.memset(d, 1.0)
            nc.scalar.activation(d, d, func)

        def ln_stats(lnp, psum_sum, psum_ssq, masked):
            m_bc = lnp.tile([128, M], F32, tag="mbc", name="mbc")
            nc.vector.tensor_scalar_mul(m_bc, psum_sum, 1.0 / D)
            mm = lnp.tile([128, M], F32, tag="mm", name="mm")
            nc.vector.tensor_mul(mm, m_bc, m_bc)
            var = lnp.tile([128, M], F32, tag="var", name="var")
            nc.vector.scalar_tensor_tensor(var, psum_ssq, 1.0 / D, mm,
                                           op0=OP.mult, op1=OP.subtract)
            sd = lnp.tile([128, M], F32, tag="sd", name="sd")
            nc.scalar.activation(sd, var, AF.Sqrt, bias=consts["epsc"][:, 0:1])
            rstd = lnp.tile([128, M], F32, tag="rstd", name="rstd")
            nc.vector.reciprocal(rstd, sd)
            if masked:
                rstd_m = lnp.tile([128, M], F32, tag="rstdm", name="rstd_m")
                nc.vector.tensor_mul(rstd_m, rstd, consts["rowmask"])
                return m_bc, rstd_m
            return m_bc, rstd

        def kv_own_cc(xTq_t, wstream, psK, psV):
            """K/V projections for the own block + bounce-out + AllGather.

            Returns the gathered DRAM tile (landed at the next rep's start).
            """
            kT_own = own.tile([128, DC, M], BF16, tag="kOwn", name="kT_own")
            vP_own = own.tile([128, 4, VW], BF16, tag="vOwn", name="vP_own")
            bnc = dram.tile([128, KVW], BF16, tag="bnc", name="bnc")
            gat = dram.tile([4, 128, KVW], BF16, tag="gat", name="gat")
            for kc in range(DC):
                wk_c = wstream.tile([128, DC, 128], BF16, tag="wks",
                                    name="wk_c")
                nc.sync.dma_start(out=wk_c,
                                  in_=wk_r[:, :, kc * 128:(kc + 1) * 128])
                pk = psK.tile([128, M], F32, tag="pk", name="pk")
                for dc in range(DC):
                    nc.tensor.matmul(pk, wk_c[:, dc, :], xTq_t[:, dc, :],
                                     start=(dc == 0), stop=(dc == DC - 1))
                nc.vector.tensor_scalar_add(kT_own[:, kc, :], pk,
                                            consts["bk"][:, kc:kc + 1])
            nc.sync.dma_start(
                out=bnc[:, 0:KW].rearrange("p (c n) -> p c n", c=DC),
                in_=kT_own)
            wv_t = wstream.tile([128, DC, VW], BF16, tag="wv", name="wv_t")
            nc.sync.dma_start(out=wv_t, in_=wv_r)
            for tq in range(4):
                for vb in range(2):
                    pv = psV.tile([128, VW // 2], F32, tag="pv", name="pv")
                    for dc in range(DC):
                        nc.tensor.matmul(
                            pv, xTq_t[:, dc, tq * 128:(tq + 1) * 128],
                            wv_t[:, dc, vb * (VW // 2):(vb + 1) * (VW // 2)],
                            start=(dc == 0), stop=False)
                    nc.tensor.matmul(
                        pv, consts["onesr"][0:1, 0:128],
                        consts["bv"][0:1, vb * (VW // 2):(vb + 1) * (VW // 2)],
                        start=False, stop=True)
                    nc.vector.tensor_scalar_mul(
                        vP_own[:, tq, vb * (VW // 2):(vb + 1) * (VW // 2)],
                        pv, consts["vmask"][:, tq:tq + 1])
            nc.sync.dma_start(
                out=bnc[:, KW:KVW].rearrange("p (c n) -> p c n", c=4),
                in_=vP_own)
            nc.gpsimd.collective_compute(
                "AllGather", mybir.AluOpType.bypass,
                replica_groups=GROUPS,
                ins=[bnc.opt()], outs=[gat.opt()])
            return gat

        # ------- prologue: first TWO reps' K/V gathers (2-deep pipe) -------
        from collections import deque
        gatq = deque()
        with tc.tile_pool(name="p0x", bufs=1) as p0x, \
             tc.tile_pool(name="p0s", bufs=3) as p0s, \
             tc.tile_pool(name="psK0", bufs=2, space="PSUM") as psK0, \
             tc.tile_pool(name="psV0", bufs=2, space="PSUM") as psV0:
            cloads()
            xTq_t0 = p0x.tile([128, DC, M], BF16, tag="xTq0", name="xTq_t0")
            for dc in range(DC):
                nc.sync.dma_start(out=xTq_t0[:, dc, :], in_=xTq_r[:, dc, :])
            gatq.append(kv_own_cc(xTq_t0, p0s, psK0, psV0))
            if reps >= 2:
                gatq.append(kv_own_cc(xTq_t0, p0s, psK0, psV0))

        for _rep in range(reps):
            kT_t = big.tile([128, 4, DC, M], BF16, tag="kT", name="kT")
            vP_t = big.tile([128, NKC, VW], BF16, tag="vP", name="vP")
            qT_t = big.tile([128, DC, M], BF16, tag="qT", name="qT")
            aT_t = big.tile([128, DC, M], BF16, tag="aT", name="aT")
            y1_t = big.tile([128, DC, M], F32R, tag="y1", name="y1")
            nT_t = big.tile([128, DC, M], BF16, tag="nT", name="nT")
            y2_t = big.tile([128, DC, M], F32R, tag="y2", name="y2")

            # ---- land gathered K/V; Q-proj; next rep's K/V + gather ----
            with tc.tile_pool(name="p1x", bufs=2) as p1x, \
                 tc.tile_pool(name="p1q", bufs=6) as p1q, \
                 tc.tile_pool(name="p1s", bufs=3) as p1s, \
                 tc.tile_pool(name="psQ", bufs=2, space="PSUM") as psQ, \
                 tc.tile_pool(name="psK", bufs=2, space="PSUM") as psK, \
                 tc.tile_pool(name="psV", bufs=2, space="PSUM") as psV:
                cloads()
                prefetch_act(p1x, "de", AF.Exp)
                xTq_t = xq2.tile([128, DC, M], BF16, tag="xTq", name="xTq_t")
                for dc in range(DC):
                    nc.sync.dma_start(out=xTq_t[:, dc, :], in_=xTq_r[:, dc, :])
                # issue all Q weight loads BEFORE the (collective-gated)
                # landing DMAs so Q compute overlaps the landing transfer
                wq_cs = []
                for qc in range(DC):
                    wq_c = p1q.tile([128, DC, 128], BF16, tag="wqs",
                                    name="wq_c")
                    nc.sync.dma_start(out=wq_c,
                                      in_=wq_r[:, :, qc * 128:(qc + 1) * 128])
                    wq_cs.append(wq_c)
                # landing is collective-gated: issue from the Act queue,
                # which is idle here and whose next real work (exp) depends
                # on the landed K/V anyway - the SP queue never blocks
                gat_cur = gatq.popleft()
                for tb in range(4):
                    nc.scalar.dma_start(
                        out=kT_t[:, tb],
                        in_=gat_cur[tb][:, 0:KW].rearrange(
                            "p (c n) -> p c n", c=DC))
                for tb in range(4):
                    nc.scalar.dma_start(
                        out=vP_t[:, tb * 4:(tb + 1) * 4],
                        in_=gat_cur[tb][:, KW:KVW].rearrange(
                            "p (c n) -> p c n", c=4))
                for qc in range(DC):
                    pq = psQ.tile([128, M], F32, tag="pq", name="pq")
                    for dc in range(DC):
                        nc.tensor.matmul(pq, wq_cs[qc][:, dc, :],
                                         xTq_t[:, dc, :],
                                         start=(dc == 0), stop=(dc == DC - 1))
                    nc.vector.tensor_scalar_add(qT_t[:, qc, :], pq,
                                                consts["bq"][:, qc:qc + 1])
                if _rep < reps - 2:
                    gatq.append(kv_own_cc(xTq_t, p1s, psK, psV))

            # -------- attention (paired-chunk exp) --------
            wproj_t = big.tile([128, DC, D], BF16, tag="wproj", name="wproj_t")
            nc.sync.dma_start(out=wproj_t, in_=wproj_r)
            with tc.tile_pool(name="attp", bufs=3) as attp, \
                 tc.tile_pool(name="atts", bufs=2) as atts, \
                 tc.tile_pool(name="psS", bufs=2, space="PSUM") as psS, \
                 tc.tile_pool(name="psB", bufs=1, space="PSUM") as psB, \
                 tc.tile_pool(name="psU", bufs=2, space="PSUM") as psU:
                for h in range(H):
                    po = (h % 2) * 64
                    chk = h // 2
                    pu = psU.tile([128, M], F32, tag="pu", name="pu")
                    for kc2 in range(NKC // 2):
                        s2 = psS.tile([128, 2, M], F32, tag="s", name="s2")
                        for j in range(2):
                            kc = kc2 * 2 + j
                            nc.tensor.matmul(
                                s2[:, j, :],
                                kT_t[po:po + 64, kc // 4, chk,
                                     (kc % 4) * 128:(kc % 4 + 1) * 128],
                                qT_t[po:po + 64, chk, :],
                                start=True, stop=True)
                        e2 = attp.tile([128, 2, M], BF16, tag="exp", name="e2")
                        nc.scalar.activation(e2, s2, AF.Exp)
                        for j in range(2):
                            kc = kc2 * 2 + j
                            nc.tensor.matmul(
                                pu[0:DH + 1, :],
                                vP_t[:, kc, h * (DH + 1):(h + 1) * (DH + 1)],
                                e2[:, j, :],
                                start=(kc == 0), stop=(kc == NKC - 1))
                    # denominator broadcast via PE; normalize on DVE
                    srow = atts.tile([1, M], F32R, tag="srow", name="srow")
                    nc.vector.tensor_copy(srow, pu[DH:DH + 1, :])
                    sbc = psB.tile([64, M], F32, tag="sbc", name="sbc")
                    nc.tensor.matmul(sbc, consts["ones64"], srow,
                                     start=True, stop=True)
                    rinv = atts.tile([64, M], F32, tag="rinv", name="rinv")
                    nc.vector.reciprocal(rinv, sbc)
                    nc.vector.tensor_mul(aT_t[po:po + 64, chk, :],
                                         pu[0:DH, :], rinv)

            # ------------ proj + residual + LN1 ------------
            with tc.tile_pool(name="p3", bufs=1) as p3, \
                 tc.tile_pool(name="p3s", bufs=2) as p3s:
                with tc.tile_pool(name="psP", bufs=2, space="PSUM") as psP, \
                     tc.tile_pool(name="psT", bufs=1, space="PSUM") as psT:
                    prefetch_act(p3, "ds", AF.Sqrt)
                    psum_sum = psT.tile([128, M], F32, tag="s1",
                                        name="psum_sum")
                    psum_ssq = psT.tile([128, M], F32, tag="s2",
                                        name="psum_ssq")
                    for do in range(DC):
                        pp = psP.tile([128, M], F32, tag="pp", name="pp")
                        for di in range(DC):
                            nc.tensor.matmul(
                                pp, wproj_t[:, di, do * 128:(do + 1) * 128],
                                aT_t[:, di, :], start=(di == 0),
                                stop=(di == DC - 1))
                        nc.vector.scalar_tensor_tensor(
                            y1_t[:, do, :], pp, consts["bprj"][:, do:do + 1],
                            xTq_t[:, do, :], op0=OP.add, op1=OP.add)
                        sq = p3s.tile([128, M], F32R, tag="sq", name="sq")
                        nc.vector.tensor_mul(sq, y1_t[:, do, :].bitcast(F32),
                                             y1_t[:, do, :].bitcast(F32))
                        nc.tensor.matmul(psum_sum, consts["ones128"],
                                         y1_t[:, do, :],
                                         start=(do == 0), stop=(do == DC - 1))
                        nc.tensor.matmul(psum_ssq, consts["ones128"], sq,
                                         start=(do == 0), stop=(do == DC - 1))
                    m_bc, rstd = ln_stats(p3, psum_sum, psum_ssq, masked=False)
                    prefetch_act(p3, "dg", AF.Gelu_apprx_tanh)
                    # nT: LN1 applied, unmasked (dead rows die at LN2 output)
                    for do in range(DC):
                        t1 = p3s.tile([128, M], F32, tag="t1", name="t1")
                        nc.vector.tensor_sub(t1, y1_t[:, do, :].bitcast(F32),
                                             m_bc)
                        t2 = p3s.tile([128, M], F32, tag="t2", name="t2")
                        nc.vector.tensor_mul(t2, t1, rstd)
                        nc.vector.tensor_scalar(nT_t[:, do, :], t2,
                                                consts["l1g"][:, do:do + 1],
                                                consts["l1b"][:, do:do + 1],
                                                op0=OP.mult, op1=OP.add)
                # b2 * rowmask for the masked LN2 tail, produced early into
                # the (now dead) y1 buffer slot; overlaps FFN matmuls.
                brm_t = big.tile([128, DC, M], F32R, tag="y1", name="brm")
                for do in range(DC):
                    nc.vector.tensor_scalar_mul(brm_t[:, do, :],
                                                consts["rowmask"],
                                                consts["l2b"][:, do:do + 1])

                # ------------ FFN + residual + LN2 ------------
                with tc.tile_pool(name="p4a", bufs=2) as p4a, \
                     tc.tile_pool(name="psM", bufs=1, space="PSUM") as psM:
                    psm = [psM.tile([128, M], F32, tag=f"m{do}",
                                    name=f"psm{do}") for do in range(DC)]
                    with tc.tile_pool(name="p4w", bufs=3) as p4w, \
                         tc.tile_pool(name="psF", bufs=2, space="PSUM") as psF:
                        for f in range(FC):
                            wfcf = p4w.tile([128, DC, 128], BF16, tag="wfcf",
                                            name="wfcf")
                            nc.sync.dma_start(
                                out=wfcf,
                                in_=wfc_r[:, :, f * 128:(f + 1) * 128])
                            woutf = p4w.tile([128, D], BF16, tag="woutf",
                                             name="woutf")
                            nc.sync.dma_start(out=woutf, in_=wout_r[:, f, :])
                            pf = psF.tile([128, M], F32, tag="pf", name="pf")
                            for dc in range(DC):
                                nc.tensor.matmul(pf, wfcf[:, dc, :],
                                                 nT_t[:, dc, :],
                                                 start=(dc == 0),
                                                 stop=(dc == DC - 1))
                            a1 = p4a.tile([128, M], BF16, tag="a1", name="a1")
                            nc.scalar.activation(a1, pf, AF.Gelu_apprx_tanh,
                                                 bias=consts["bfc"][:, f:f + 1])
                            for do in range(DC):
                                nc.tensor.matmul(
                                    psm[do], woutf[:, do * 128:(do + 1) * 128],
                                    a1, start=(f == 0), stop=(f == FC - 1))
                        prefetch_act(p4a, "ds2", AF.Sqrt)

                    with tc.tile_pool(name="psT2", bufs=1,
                                      space="PSUM") as psT2:
                        psum_sum2 = psT2.tile([128, M], F32, tag="s1",
                                              name="psum_sum2")
                        psum_ssq2 = psT2.tile([128, M], F32, tag="s2",
                                              name="psum_ssq2")
                        for do in range(DC):
                            nc.vector.scalar_tensor_tensor(
                                y2_t[:, do, :], psm[do],
                                consts["bout"][:, do:do + 1],
                                nT_t[:, do, :], op0=OP.add, op1=OP.add)
                            sq2 = p4a.tile([128, M], F32R, tag="sq2",
                                           name="sq2")
                            nc.vector.tensor_mul(sq2,
                                                 y2_t[:, do, :].bitcast(F32),
                                                 y2_t[:, do, :].bitcast(F32))
                            nc.tensor.matmul(psum_sum2, consts["ones128"],
                                             y2_t[:, do, :],
                                             start=(do == 0),
                                             stop=(do == DC - 1))
                            nc.tensor.matmul(psum_ssq2, consts["ones128"],
                                             sq2, start=(do == 0),
                                             stop=(do == DC - 1))
                        m2, rstd2m = ln_stats(p3, psum_sum2, psum_ssq2,
                                              masked=True)
                        for do in range(DC):
                            t1 = p3s.tile([128, M], F32, tag="u1", name="u1")
                            nc.vector.tensor_sub(
                                t1, y2_t[:, do, :].bitcast(F32), m2)
                            t2 = p3s.tile([128, M], F32, tag="u2", name="u2")
                            nc.vector.tensor_mul(t2, t1, rstd2m)
                            hc = p3s.tile([128, M], F32, tag="hc", name="hc")
                            nc.vector.scalar_tensor_tensor(
                                hc, t2, consts["l2g"][:, do:do + 1],
                                brm_t[:, do, :].bitcast(F32),
                                op0=OP.mult, op1=OP.add)
                            nc.sync.dma_start(out=hT_r[do], in_=hc)

    nc.compile()
    return nc


def _shared_arrays(inputs):
    import ml_dtypes
    f32 = np.float32
    bf16 = ml_dtypes.bfloat16
    w_qkv = np.ascontiguousarray(inputs["w_qkv"], dtype=f32)
    b_qkv = np.ascontiguousarray(inputs["b_qkv"], dtype=f32)

    def pc(v):  # [C*128] -> [128, C] column-chunk layout
        v = np.ascontiguousarray(v, dtype=f32)
        return np.ascontiguousarray(v.reshape(-1, 128).T)

    wv_ext = np.zeros((D, VW), f32)
    bv_ext = np.zeros((1, VW), f32)
    for h in range(H):
        wv_ext[:, h * (DH + 1):h * (DH + 1) + DH] = \
            w_qkv[:, 2 * D + h * DH:2 * D + (h + 1) * DH]
        bv_ext[0, h * (DH + 1):h * (DH + 1) + DH] = \
            b_qkv[2 * D + h * DH:2 * D + (h + 1) * DH]
        bv_ext[0, h * (DH + 1) + DH] = 1.0

    return dict(
        wq=w_qkv[:, 0:D].astype(bf16),
        bq_pc=pc(b_qkv[0:D]),
        wk=w_qkv[:, D:2 * D].astype(bf16),
        bk_pc=pc(b_qkv[D:2 * D]),
        wv=wv_ext.astype(bf16),
        bv=bv_ext.astype(bf16),
        wproj=np.asarray(inputs["w_proj"], f32).astype(bf16),
        wfc=np.asarray(inputs["w_fc"], f32).astype(bf16),
        wout=np.asarray(inputs["w_out"], f32).astype(bf16),
        onesr=np.ones((1, M), bf16),
        epsc=np.full((128, 1), EPS, f32),
        ones128=np.ones((128, 128), f32),
        ones64=np.ones((1, 64), f32),
        bprj=pc(inputs["b_proj"]),
        bfc=pc(inputs["b_fc"]),
        bout=pc(inputs["b_out"]),
        l1g=pc(inputs["ln1_g"]),
        l1b=pc(inputs["ln1_b"]),
        l2g=pc(inputs["ln2_g"]),
        l2b=pc(inputs["ln2_b"]),
    )


def make_in_maps(inputs):
    import ml_dtypes
    bf16 = ml_dtypes.bfloat16
    inputs = {k: np.asarray(v) for k, v in inputs.items()}
    x = np.ascontiguousarray(inputs["x"], dtype=np.float32)
    lengths = np.asarray(inputs["lengths"]).astype(np.int64)
    shared = _shared_arrays(inputs)
    pos = np.arange(T)
    in_maps = []
    for c in range(8):
        b, r = divmod(c, 4)
        sl = slice(r * M, (r + 1) * M)
        xTb = np.ascontiguousarray(x[b].T)
        km = (pos[sl] < lengths[b]).astype(np.float32)
        m = dict(shared)
        m["xTq"] = np.ascontiguousarray(xTb[:, sl]).astype(bf16)
        m["vmask"] = np.ascontiguousarray(km.reshape(4, 128).T)
        m["rowmask"] = np.ascontiguousarray(
            np.broadcast_to(km[None, :], (128, M)))
        in_maps.append(m)
    return in_maps


def get_program(reps=1):
    key = f"nc{reps}"
    if key not in _STATE:
        _STATE[key] = _build_program(reps)
    return _STATE[key]


def kernel(**inputs) -> np.ndarray:
    from concourse.bass_utils import run_bass_kernel_spmd

    nc = get_program()
    in_maps = make_in_maps(inputs)
    res = run_bass_kernel_spmd(nc, in_maps, list(range(8)), trace=False)
    out = np.zeros((B, T, D), np.float32)
    for c in range(8):
        b, r = divmod(c, 4)
        out[b, r * M:(r + 1) * M, :] = res.results[c]["hT"].T
    return out
